# revision 19
# baseline (speedup 1.0000x reference)
"""BitLinear (RMSNorm + ternary-quantized linear) on 8 trn2 NeuronCores.

Reference math (fp32):
    xn   = x * rsqrt(mean(x^2, -1) + 1e-5) * gamma          # [B,S,K]
    s    = max(mean(|w|), 1e-5)                             # scalar
    q    = round(clip(w / s, -1, 1))                        # ternary {-1,0,1}
    out  = (xn @ q.T) * s                                   # [B,S,Dout]

Identities used by the kernel:
    q  = (w > s/2) - (w < -s/2)  (exact, incl. round-half-even at |wn|=0.5)
    2q = Sign(w - s/2) + Sign(w + s/2), EXCEPT at exact ties w == +-s/2
         where the pair gives +-1; thresholding the pair at +-1.5 maps
         {-2,-1,0,1,2} -> {-1,0,0,0,1} which is exact everywhere.
    out[t,o] = inv[t] * sum_k x[t,k] * (gamma[k]*s*q[o,k])
gamma*s folds into the quantized weight (still exactly +-gamma*s/0 in
fp16) and the epilogue scale is the pure per-token inv. The contraction
runs on the PE in fp16 with fp32 PSUM accumulation.

The ternary pattern is exquisitely sensitive to s: one weight flipped
across the s/2 threshold costs ~1.6e-2 of the 2e-2 rel-err budget
(max-err metric), and this dataset contains at least one exact tie.
So comparisons use f32 w against the exact full-weight |w|-mean,
reduced with the same instruction chain on the same slices as the
validated baseline (w_red 1/8 per core + AllReduce), with tie-exact
combines.

Collective cost control: a dependency-free dummy AllReduce is issued at
t~0 so the runtime's one-time pre-collective barrier (~50us) runs
concurrently with the w_red scan; the real AllReduce follows as soon as
the partials land. Pre-s loads ride the sync/HWDGE queue so the SWDGE
ring the collectives drain stays empty; every post-s gpsimd DMA is
emitted after the `allv` copy and therefore stays quiet until the
collective completes.

x ships as fp16 (host cast; noise-like 2.4e-4, no pattern risk): loaded
once [tok,k] for RMSNorm stats (ACT square+accum -> Rsqrt) and once via
xbar DMA-transpose directly from DRAM into xT[p, t, tok], k = t*128+p.
Epilogue scale-copies run on DVE (idle after quantize) so the ACT queue
never backpressures PSUM.

Sharding: 2 token-groups x 4 dout-groups (core = rg*4 + cg).
"""

import numpy as np

import concourse.bass as bass
import concourse.tile as tile
from concourse import bacc, mybir
from concourse.bass_utils import run_bass_kernel_spmd

F32 = mybir.dt.float32
F16 = mybir.dt.float16
BF16 = mybir.dt.bfloat16
F8 = mybir.dt.float8e4

# Full-problem constants
B, S, K, DOUT = 4, 2048, 2048, 8192
N_CORES = 8
RG, CG = 2, 4  # token groups x dout groups
TOK_SH = (B * S) // RG      # 4096 tokens per core
DOUT_SH = DOUT // CG        # 2048 out-features per core
RED_ROWS = DOUT // N_CORES  # 1024 rows of w reduced per core for mean(|w|)
W_COUNT = float(DOUT * K)
EPS = 1e-5


def build_nc(tok_sh=TOK_SH, k=K, dout_sh=DOUT_SH, n_cores=N_CORES,
             strip_blk=4, use_cc=True):
    """Build the SPMD Bass program (one program, per-core inputs differ)."""
    kt = k // 128            # contraction tiles
    n_strip = tok_sh // 128  # token strips
    n_wtile = dout_sh // 128
    n_blk = n_strip // strip_blk
    n_kq = max(1, kt // 4)
    kq = kt // n_kq
    n_rtile = RED_ROWS // 128
    dve_tiles = 4            # k-tiles quantized on the DVE path (rest: ACT)
    pre_strips = 12          # strips fully prepped before the main loop

    nc = bacc.Bacc("TRN2", target_bir_lowering=False, num_devices=n_cores)

    x_d = nc.declare_dram_parameter("x_sh", [tok_sh, k], F16, isOutput=False)
    w_d = nc.declare_dram_parameter("w_shT", [k, dout_sh], F32, isOutput=False)
    wr_d = nc.declare_dram_parameter("w_red", [RED_ROWS, k], F32,
                                     isOutput=False)
    g_d = nc.declare_dram_parameter("gamma", [k], F32, isOutput=False)
    out_d = nc.declare_dram_parameter("out_sh", [tok_sh, dout_sh], F32,
                                      isOutput=True)

    with tile.TileContext(nc, num_cores=n_cores) as tc:
        with (
            tc.tile_pool(name="consts", bufs=1) as consts,
            tc.tile_pool(name="f32s", bufs=1) as f32s,
            tc.tile_pool(name="f16s", bufs=1) as f16s,
            tc.tile_pool(name="qt", bufs=1) as qtp,
            tc.tile_pool(name="outp", bufs=1) as outp,
            tc.tile_pool(name="psum", bufs=8, space="PSUM") as psum,
            tc.tile_pool(name="dram", bufs=1, space="DRAM") as dram,
        ):
            # ---- constants -------------------------------------------------
            cblock = consts.tile([128, 6 + n_rtile], F32)
            ones_col = cblock[:, 0:1]
            eps_t = cblock[:, 1:2]
            prev = cblock[:, 2:3]
            allv = cblock[:, 3:4]
            junk = cblock[:, 4:5]
            eps4k_t = cblock[:, 5:6]
            parts = cblock[:, 6:6 + n_rtile]
            nc.vector.memset(ones_col, 1.0)
            nc.vector.memset(eps_t, EPS)
            nc.vector.memset(eps4k_t, EPS * 4096.0)
            ones_row = consts.tile([1, 128], F32)
            nc.vector.memset(ones_row, 1.0)

            # dummy collective: absorbs the one-time pre-collective barrier
            # while the w_red scan streams in
            if use_cc:
                cc1_in = dram.tile([128, 1], F32)
                cc1_out = dram.tile([128, 1], F32, addr_space="Shared")
                nc.gpsimd.dma_start(out=cc1_in, in_=eps_t)
                nc.gpsimd.collective_compute(
                    "AllReduce", mybir.AluOpType.add,
                    replica_groups=[list(range(n_cores))],
                    ins=[cc1_in.opt()], outs=[cc1_out.opt()],
                )

            # gamma transposed to [p, t] with k = t*128 + p
            gT = consts.tile([128, kt], F32)
            g_ap = bass.AP(tensor=g_d, offset=0, ap=[[1, 128], [128, kt]])
            nc.sync.dma_start(out=gT, in_=g_ap)
            # sblock cols: 0 s_mean, 1 s_clip, 2 s_bc, 3 t_bc, 4 nt_bc
            sblock = consts.tile([128, 7], F32)
            s_bc = sblock[:, 2:3]
            t_bc = sblock[:, 3:4]
            nt_bc = sblock[:, 4:5]
            s64 = sblock[:, 5:6]
            ns64 = sblock[:, 6:7]
            gs = consts.tile([128, kt], F32)     # gamma * s
            ngs = consts.tile([128, kt], F32)    # -gamma * s
            invb = consts.tile([128, n_strip], F32)  # per-strip 1/rms

            # ---- phase S-pre: |w| partials on sync, AllReduce on gpsimd ----
            # Identical slices + reduce chain as the validated baseline so s
            # is bit-identical (the ternary pattern must not flip).
            wts = {}
            for i in range(n_rtile):
                wrt = f32s.tile([128, k], F32, tag="wt", bufs=9,
                                name=f"wr{i}")
                nc.sync.dma_start(out=wrt,
                                  in_=wr_d[i * 128:(i + 1) * 128, :])
                nc.vector.tensor_reduce(
                    parts[:, i:i + 1], wrt, axis=mybir.AxisListType.X,
                    op=mybir.AluOpType.add, apply_absolute_value=True)
            nc.vector.tensor_reduce(prev, parts, axis=mybir.AxisListType.X,
                                    op=mybir.AluOpType.add)
            cc_in = dram.tile([128, 1], F32)
            cc_out = dram.tile([128, 1], F32, addr_space="Shared")
            nc.gpsimd.dma_start(out=cc_in, in_=prev)
            if use_cc:
                nc.gpsimd.collective_compute(
                    "AllReduce", mybir.AluOpType.add,
                    replica_groups=[list(range(n_cores))],
                    ins=[cc_in.opt()], outs=[cc_out.opt()],
                )
            else:
                nc.gpsimd.dma_start(out=cc_out, in_=cc_in)

            # ---- phase W-pre: f32 w k-tiles 0..8 resident (sync queue) -----
            for i in range(9):
                wts[i] = f32s.tile([128, dout_sh], F32, tag="wt", bufs=9,
                                   name=f"wq{i}")
                nc.sync.dma_start(out=wts[i],
                                  in_=w_d[i * 128:(i + 1) * 128, :])

            # ---- strip-prep DMAs (s-independent; compute ops come after
            # the quantize emission so the ACT/DVE FIFOs serve s first) -----
            xT_tiles = {}
            xs_tiles = {}
            x8_tiles = {}

            def prep_dma(j, pre):
                xs = f16s.tile([128, k], F16, tag="xs", bufs=2, name=f"xs{j}")
                dma = nc.sync.dma_start if pre else nc.gpsimd.dma_start
                dma(out=xs, in_=x_d[j * 128:(j + 1) * 128, :])
                xs_tiles[j] = xs
                xT = f16s.tile([128, kt, 128], F16, tag="xT", bufs=8,
                               name=f"xT{j}")
                nc.sync.dma_start_transpose(out=xT, in_=xs)
                xT_tiles[j] = xT
                x8 = f16s.tile([128, 4, 128], F8, tag="x8", bufs=12,
                               name=f"x8_{j}")
                for u in range(4):
                    nc.vector.tensor_scalar(
                        x8[:, u, :], xT[:, 12 + u, :],
                        gT[:, 12 + u:13 + u], None, mybir.AluOpType.mult)
                x8_tiles[j] = x8

            def prep_compute(j):
                xs = xs_tiles.pop(j)
                xsq = f16s.tile([128, k], BF16, tag="xsq", bufs=1,
                                name=f"xsq{j}")
                sc = f16s.tile([128, 2], F32, tag="sc", bufs=3, name=f"sc{j}")
                ssq, rms = sc[:, 0:1], sc[:, 1:2]
                nc.scalar.activation(xsq, xs,
                                     mybir.ActivationFunctionType.Square,
                                     accum_out=ssq)
                nc.scalar.activation(rms, ssq,
                                     mybir.ActivationFunctionType.Sqrt,
                                     bias=eps4k_t, scale=4096.0 / k)
                nc.vector.reciprocal(invb[:, j:j + 1], rms)

            for j in range(pre_strips):
                prep_dma(j, pre=True)

            # ---- phase S-post: finish s after the AllReduce ----------------
            # allv on the gpsimd DMA queue: every gpsimd DMA emitted after it
            # stays quiet until the collective completes.
            nc.gpsimd.dma_start(out=allv, in_=cc_out)
            if use_cc:
                nc.gpsimd.dma_start(out=junk, in_=cc1_out)
            tot_ps = psum.tile([1, 1], F32, tag="mm")
            nc.tensor.matmul(tot_ps, lhsT=allv, rhs=ones_col,
                             start=True, stop=True)
            nc.scalar.activation(sblock[0:1, 0:1], tot_ps,
                                 mybir.ActivationFunctionType.Copy,
                                 scale=1.0 / W_COUNT)
            nc.vector.tensor_scalar_max(sblock[0:1, 1:2], sblock[0:1, 0:1],
                                        EPS)
            s_bc_ps = psum.tile([128, 1], F32, tag="mm")
            nc.tensor.matmul(s_bc_ps, lhsT=ones_row, rhs=sblock[0:1, 1:2],
                             start=True, stop=True)
            nc.scalar.copy(s_bc, s_bc_ps)
            nc.scalar.mul(t_bc, s_bc, 0.5)
            nc.scalar.mul(nt_bc, s_bc, -0.5)
            nc.vector.tensor_scalar(gs, gT, s_bc, 64.0,
                                    mybir.AluOpType.mult,
                                    mybir.AluOpType.mult)
            nc.vector.tensor_scalar(ngs, gs, -1.0, None, mybir.AluOpType.mult)
            nc.vector.tensor_scalar(s64, s_bc, 64.0, None,
                                    mybir.AluOpType.mult)
            nc.vector.tensor_scalar(ns64, s64, -1.0, None,
                                    mybir.AluOpType.mult)

            # ---- quantize --------------------------------------------------
            # qQ[q][p, u, o] = gamma*s*q(w[o, (kq*q+u)*128+p]) in fp16
            qQs = [qtp.tile([128, kq, dout_sh], F16, tag=f"qQ{q}",
                            name=f"qQ{q}") for q in range(3)]
            q8s = [qtp.tile([128, 2, dout_sh], F8, tag=f"q8_{p}",
                            name=f"q8_{p}") for p in range(2)]

            def qslice(i):
                if i < 12:
                    return qQs[i // kq][:, i % kq, :]
                return q8s[(i - 12) // 2][:, (i - 12) % 2, :]

            for i in range(kt):
                if i not in wts:
                    wts[i] = f32s.tile([128, dout_sh], F32, tag="wt", bufs=9,
                                       name=f"wq{i}")
                    nc.gpsimd.dma_start(out=wts[i],
                                        in_=w_d[i * 128:(i + 1) * 128, :])
                wt = wts[i]
                # DVE 3-op: (w>t)*scale + (w<-t)*(-scale); exact at ties.
                # Tiles 12-15 go to fp8 (+-fl8(s*64)); gamma rides in x8.
                f8_tile = i >= 12
                odt = F8 if f8_tile else F16
                ptag, ntag = ("pos8", "nm8") if f8_tile else ("pos", "nm")
                psc = s64 if f8_tile else gs[:, i:i + 1]
                nsc = ns64 if f8_tile else ngs[:, i:i + 1]
                pos = f16s.tile([128, dout_sh], odt, tag=ptag, bufs=2,
                                name=f"pos{i}")
                nc.vector.tensor_scalar(pos, wt, t_bc, psc,
                                        mybir.AluOpType.is_gt,
                                        mybir.AluOpType.mult)
                nm = f16s.tile([128, dout_sh], odt, tag=ntag, bufs=2,
                               name=f"nm{i}")
                nc.vector.tensor_scalar(nm, wt, nt_bc, nsc,
                                        mybir.AluOpType.is_lt,
                                        mybir.AluOpType.mult)
                nc.vector.tensor_tensor(qslice(i), pos, nm,
                                        mybir.AluOpType.add)

            # stats for the pre-loaded strips (after quantize in FIFO order)
            for j in range(pre_strips):
                prep_compute(j)

            # ---- main loop -------------------------------------------------
            def chain_chunk(j, d, ps, qtr):
                if qtr == 3:
                    for p in range(2):
                        nc.tensor.matmul(
                            ps, lhsT=x8_tiles[j][:, 2 * p:2 * p + 2, :],
                            rhs=q8s[p][:, :, d * 512:(d + 1) * 512],
                            start=False, stop=(p == 1),
                            perf_mode=mybir.MatmulPerfMode.DoubleRow)
                    return
                for u in range(kq):
                    t = qtr * kq + u
                    nc.tensor.matmul(
                        ps, lhsT=xT_tiles[j][:, t, :],
                        rhs=qQs[qtr][:, u, d * 512:(d + 1) * 512],
                        start=(t == 0), stop=False)

            def finish_psum_tile(j, d, ps):
                ob = outp.tile([128, 512], F32, tag="ob", bufs=2,
                               name=f"ob{j}_{d}")
                nc.vector.tensor_scalar(ob, ps, invb[:, j:j + 1], None,
                                        mybir.AluOpType.mult)
                nc.scalar.dma_start(
                    out=out_d[j * 128:(j + 1) * 128, d * 512:(d + 1) * 512],
                    in_=ob)

            def emit_psum_tile(j, d):
                ps = psum.tile([128, 512], F32, tag="mm", name=f"ps{j}_{d}")
                for qtr in range(n_kq):
                    chain_chunk(j, d, ps, qtr)
                finish_psum_tile(j, d, ps)

            # block 0: quarter-interleaved across all 8 banks, 2 waves
            for wave in range(2):
                tiles = [(j, d) for d in (2 * wave, 2 * wave + 1)
                         for j in range(strip_blk)]
                pss = {(j, d): psum.tile([128, 512], F32, tag="mm",
                                         name=f"ps{j}_{d}")
                       for (j, d) in tiles}
                for qtr in range(n_kq):
                    for (j, d) in tiles:
                        chain_chunk(j, d, pss[(j, d)], qtr)
                for (j, d) in tiles:
                    finish_psum_tile(j, d, pss[(j, d)])

            # blocks 1..: plain deep-pipelined chains, preps dripped in
            next_prep = pre_strips
            for b in range(1, n_blk):
                for d in range(n_wtile // 4):
                    for j in range(b * strip_blk, (b + 1) * strip_blk):
                        emit_psum_tile(j, d)
                    if next_prep < n_strip:
                        prep_dma(next_prep, pre=False)
                        prep_compute(next_prep)
                        next_prep += 1

    nc.compile()
    return nc


_NC_CACHE = {}


def _get_nc():
    if "nc" not in _NC_CACHE:
        _NC_CACHE["nc"] = build_nc()
    return _NC_CACHE["nc"]


def make_in_maps(x, weight, gamma):
    """Shard + lay out host-side. x:[B,S,K] f32, weight:[DOUT,K] f32."""
    x = np.asarray(x, dtype=np.float32)
    weight = np.ascontiguousarray(np.asarray(weight, dtype=np.float32))
    gamma = np.ascontiguousarray(np.asarray(gamma, dtype=np.float32))

    x16 = x.reshape(B * S, K).astype(np.float16)
    wT = np.ascontiguousarray(weight.T)  # [K, DOUT] f32
    in_maps = []
    for c in range(N_CORES):
        rg, cg = c // CG, c % CG
        in_maps.append({
            "x_sh": np.ascontiguousarray(
                x16[rg * TOK_SH:(rg + 1) * TOK_SH]),
            "w_shT": np.ascontiguousarray(
                wT[:, cg * DOUT_SH:(cg + 1) * DOUT_SH]),
            "w_red": weight[c * RED_ROWS:(c + 1) * RED_ROWS],
            "gamma": gamma,
        })
    return in_maps


def kernel(x, weight, gamma):
    in_maps = make_in_maps(x, weight, gamma)
    nc = _get_nc()
    res = run_bass_kernel_spmd(nc, in_maps, list(range(N_CORES))).results

    out = np.empty((B * S, DOUT), dtype=np.float32)
    for c in range(N_CORES):
        rg, cg = c // CG, c % CG
        out[rg * TOK_SH:(rg + 1) * TOK_SH,
            cg * DOUT_SH:(cg + 1) * DOUT_SH] = res[c]["out_sh"]
    return out.reshape(B, S, DOUT)


# revision 20
# speedup vs baseline: 1.0295x; 1.0295x over previous
"""BitLinear (RMSNorm + ternary-quantized linear) on 8 trn2 NeuronCores.

Reference math (fp32):
    xn   = x * rsqrt(mean(x^2, -1) + 1e-5) * gamma          # [B,S,K]
    s    = max(mean(|w|), 1e-5)                             # scalar
    q    = round(clip(w / s, -1, 1))                        # ternary {-1,0,1}
    out  = (xn @ q.T) * s                                   # [B,S,Dout]

Identities used by the kernel:
    q  = (w > s/2) - (w < -s/2)  (exact, incl. round-half-even at |wn|=0.5)
    2q = Sign(w - s/2) + Sign(w + s/2), EXCEPT at exact ties w == +-s/2
         where the pair gives +-1; thresholding the pair at +-1.5 maps
         {-2,-1,0,1,2} -> {-1,0,0,0,1} which is exact everywhere.
    out[t,o] = inv[t] * sum_k x[t,k] * (gamma[k]*s*q[o,k])
gamma*s folds into the quantized weight (still exactly +-gamma*s/0 in
fp16) and the epilogue scale is the pure per-token inv. The contraction
runs on the PE in fp16 with fp32 PSUM accumulation.

The ternary pattern is exquisitely sensitive to s: one weight flipped
across the s/2 threshold costs ~1.6e-2 of the 2e-2 rel-err budget
(max-err metric), and this dataset contains at least one exact tie.
So comparisons use f32 w against the exact full-weight |w|-mean,
reduced with the same instruction chain on the same slices as the
validated baseline (w_red 1/8 per core + AllReduce), with tie-exact
combines.

Collective cost control: a dependency-free dummy AllReduce is issued at
t~0 so the runtime's one-time pre-collective barrier (~50us) runs
concurrently with the w_red scan; the real AllReduce follows as soon as
the partials land. Pre-s loads ride the sync/HWDGE queue so the SWDGE
ring the collectives drain stays empty; every post-s gpsimd DMA is
emitted after the `allv` copy and therefore stays quiet until the
collective completes.

x ships as fp16 (host cast; noise-like 2.4e-4, no pattern risk): loaded
once [tok,k] for RMSNorm stats (ACT square+accum -> Rsqrt) and once via
xbar DMA-transpose directly from DRAM into xT[p, t, tok], k = t*128+p.
Epilogue scale-copies run on DVE (idle after quantize) so the ACT queue
never backpressures PSUM.

Sharding: 2 token-groups x 4 dout-groups (core = rg*4 + cg).
"""

import numpy as np

import concourse.bass as bass
import concourse.tile as tile
from concourse import bacc, mybir
from concourse.bass_utils import run_bass_kernel_spmd

F32 = mybir.dt.float32
F16 = mybir.dt.float16
BF16 = mybir.dt.bfloat16

# Full-problem constants
B, S, K, DOUT = 4, 2048, 2048, 8192
N_CORES = 8
RG, CG = 2, 4  # token groups x dout groups
TOK_SH = (B * S) // RG      # 4096 tokens per core
DOUT_SH = DOUT // CG        # 2048 out-features per core
RED_ROWS = DOUT // N_CORES  # 1024 rows of w reduced per core for mean(|w|)
W_COUNT = float(DOUT * K)
EPS = 1e-5


def build_nc(tok_sh=TOK_SH, k=K, dout_sh=DOUT_SH, n_cores=N_CORES,
             strip_blk=4, use_cc=True):
    """Build the SPMD Bass program (one program, per-core inputs differ)."""
    kt = k // 128            # contraction tiles
    n_strip = tok_sh // 128  # token strips
    n_wtile = dout_sh // 128
    n_blk = n_strip // strip_blk
    n_kq = max(1, kt // 4)
    kq = kt // n_kq
    n_rtile = RED_ROWS // 128
    dve_tiles = 4            # k-tiles quantized on the DVE path (rest: ACT)
    pre_strips = 12          # strips fully prepped before the main loop

    nc = bacc.Bacc("TRN2", target_bir_lowering=False, num_devices=n_cores)

    x_d = nc.declare_dram_parameter("x_sh", [tok_sh, k], F16, isOutput=False)
    w_d = nc.declare_dram_parameter("w_shT", [k, dout_sh], F32, isOutput=False)
    wr_d = nc.declare_dram_parameter("w_red", [RED_ROWS, k], F32,
                                     isOutput=False)
    g_d = nc.declare_dram_parameter("gamma", [k], F32, isOutput=False)
    out_d = nc.declare_dram_parameter("out_sh", [tok_sh, dout_sh], F32,
                                      isOutput=True)

    with tile.TileContext(nc, num_cores=n_cores) as tc:
        with (
            tc.tile_pool(name="consts", bufs=1) as consts,
            tc.tile_pool(name="f32s", bufs=1) as f32s,
            tc.tile_pool(name="f16s", bufs=1) as f16s,
            tc.tile_pool(name="qt", bufs=1) as qtp,
            tc.tile_pool(name="outp", bufs=1) as outp,
            tc.tile_pool(name="psum", bufs=8, space="PSUM") as psum,
            tc.tile_pool(name="dram", bufs=1, space="DRAM") as dram,
        ):
            # ---- constants -------------------------------------------------
            cblock = consts.tile([128, 5 + n_rtile], F32)
            ones_col = cblock[:, 0:1]
            eps_t = cblock[:, 1:2]
            prev = cblock[:, 2:3]
            allv = cblock[:, 3:4]
            junk = cblock[:, 4:5]
            parts = cblock[:, 5:5 + n_rtile]
            nc.vector.memset(ones_col, 1.0)
            nc.vector.memset(eps_t, EPS)
            ones_row = consts.tile([1, 128], F32)
            nc.vector.memset(ones_row, 1.0)

            # dummy collective: absorbs the one-time pre-collective barrier
            # while the w_red scan streams in
            if use_cc:
                cc1_in = dram.tile([128, 1], F32)
                cc1_out = dram.tile([128, 1], F32, addr_space="Shared")
                nc.gpsimd.dma_start(out=cc1_in, in_=eps_t)
                nc.gpsimd.collective_compute(
                    "AllReduce", mybir.AluOpType.add,
                    replica_groups=[list(range(n_cores))],
                    ins=[cc1_in.opt()], outs=[cc1_out.opt()],
                )

            # gamma transposed to [p, t] with k = t*128 + p
            gT = consts.tile([128, kt], F32)
            g_ap = bass.AP(tensor=g_d, offset=0, ap=[[1, 128], [128, kt]])
            nc.sync.dma_start(out=gT, in_=g_ap)
            # sblock cols: 0 s_mean, 1 s_clip, 2 s_bc, 3 t_bc, 4 nt_bc
            sblock = consts.tile([128, 5], F32)
            s_bc = sblock[:, 2:3]
            t_bc = sblock[:, 3:4]
            nt_bc = sblock[:, 4:5]
            gs = consts.tile([128, kt], F32)     # gamma * s
            ngs = consts.tile([128, kt], F32)    # -gamma * s
            invb = consts.tile([128, n_strip], F32)  # per-strip 1/rms

            # ---- phase S-pre: |w| partials on sync, AllReduce on gpsimd ----
            # Identical slices + reduce chain as the validated baseline so s
            # is bit-identical (the ternary pattern must not flip).
            wts = {}
            for i in range(n_rtile):
                wrt = f32s.tile([128, k], F32, tag="wt", bufs=9,
                                name=f"wr{i}")
                nc.sync.dma_start(out=wrt,
                                  in_=wr_d[i * 128:(i + 1) * 128, :])
                nc.vector.tensor_reduce(
                    parts[:, i:i + 1], wrt, axis=mybir.AxisListType.X,
                    op=mybir.AluOpType.add, apply_absolute_value=True)
            nc.vector.tensor_reduce(prev, parts, axis=mybir.AxisListType.X,
                                    op=mybir.AluOpType.add)
            cc_in = dram.tile([128, 1], F32)
            cc_out = dram.tile([128, 1], F32, addr_space="Shared")
            nc.gpsimd.dma_start(out=cc_in, in_=prev)
            if use_cc:
                nc.gpsimd.collective_compute(
                    "AllReduce", mybir.AluOpType.add,
                    replica_groups=[list(range(n_cores))],
                    ins=[cc_in.opt()], outs=[cc_out.opt()],
                )
            else:
                nc.gpsimd.dma_start(out=cc_out, in_=cc_in)

            # ---- phase W-pre: f32 w k-tiles 0..8 resident (sync queue) -----
            for i in range(9):
                wts[i] = f32s.tile([128, dout_sh], F32, tag="wt", bufs=9,
                                   name=f"wq{i}")
                nc.sync.dma_start(out=wts[i],
                                  in_=w_d[i * 128:(i + 1) * 128, :])

            # ---- strip-prep DMAs (s-independent; compute ops come after
            # the quantize emission so the ACT/DVE FIFOs serve s first) -----
            xT_tiles = {}
            xs_tiles = {}

            def prep_dma(j, pre):
                xs = f16s.tile([128, k], F16, tag="xs", bufs=2, name=f"xs{j}")
                dma = nc.sync.dma_start if pre else nc.gpsimd.dma_start
                dma(out=xs, in_=x_d[j * 128:(j + 1) * 128, :])
                xs_tiles[j] = xs
                xT = f16s.tile([128, kt, 128], F16, tag="xT", bufs=8,
                               name=f"xT{j}")
                nc.sync.dma_start_transpose(out=xT, in_=xs)
                xT_tiles[j] = xT

            def prep_compute(j):
                xs = xs_tiles.pop(j)
                xsq = f16s.tile([128, k], BF16, tag="xsq", bufs=1,
                                name=f"xsq{j}")
                sc = f16s.tile([128, 2], F32, tag="sc", bufs=3, name=f"sc{j}")
                ssq, rms = sc[:, 0:1], sc[:, 1:2]
                nc.scalar.activation(xsq, xs,
                                     mybir.ActivationFunctionType.Square,
                                     accum_out=ssq)
                nc.scalar.activation(rms, ssq,
                                     mybir.ActivationFunctionType.Sqrt,
                                     bias=eps_t, scale=1.0 / k)
                nc.vector.reciprocal(invb[:, j:j + 1], rms)

            for j in range(pre_strips):
                prep_dma(j, pre=True)

            # ---- phase S-post: finish s after the AllReduce ----------------
            # allv on the gpsimd DMA queue: every gpsimd DMA emitted after it
            # stays quiet until the collective completes.
            nc.gpsimd.dma_start(out=allv, in_=cc_out)
            if use_cc:
                nc.gpsimd.dma_start(out=junk, in_=cc1_out)
            tot_ps = psum.tile([1, 1], F32, tag="mm")
            nc.tensor.matmul(tot_ps, lhsT=allv, rhs=ones_col,
                             start=True, stop=True)
            nc.scalar.activation(sblock[0:1, 0:1], tot_ps,
                                 mybir.ActivationFunctionType.Copy,
                                 scale=1.0 / W_COUNT)
            nc.vector.tensor_scalar_max(sblock[0:1, 1:2], sblock[0:1, 0:1],
                                        EPS)
            s_bc_ps = psum.tile([128, 1], F32, tag="mm")
            nc.tensor.matmul(s_bc_ps, lhsT=ones_row, rhs=sblock[0:1, 1:2],
                             start=True, stop=True)
            nc.scalar.copy(s_bc, s_bc_ps)
            nc.scalar.mul(t_bc, s_bc, 0.5)
            nc.scalar.mul(nt_bc, s_bc, -0.5)
            nc.vector.tensor_scalar(gs, gT, s_bc, None, mybir.AluOpType.mult)
            nc.vector.tensor_scalar(ngs, gs, -1.0, None, mybir.AluOpType.mult)

            # ---- quantize --------------------------------------------------
            # qQ[q][p, u, o] = gamma*s*q(w[o, (kq*q+u)*128+p]) in fp16
            qQs = [qtp.tile([128, kq, dout_sh], F16, tag=f"qQ{q}",
                            name=f"qQ{q}") for q in range(n_kq)]

            def qslice(i):
                return qQs[i // kq][:, i % kq, :]

            for i in range(kt):
                if i not in wts:
                    wts[i] = f32s.tile([128, dout_sh], F32, tag="wt", bufs=9,
                                       name=f"wq{i}")
                    nc.gpsimd.dma_start(out=wts[i],
                                        in_=w_d[i * 128:(i + 1) * 128, :])
                wt = wts[i]
                # DVE 3-op: (w>t)*gs + (w<-t)*(-gs); exact at ties
                pos = f16s.tile([128, dout_sh], F16, tag="pos", bufs=2,
                                name=f"pos{i}")
                nc.vector.tensor_scalar(pos, wt, t_bc, gs[:, i:i + 1],
                                        mybir.AluOpType.is_gt,
                                        mybir.AluOpType.mult)
                nm = f16s.tile([128, dout_sh], F16, tag="nm", bufs=2,
                               name=f"nm{i}")
                nc.vector.tensor_scalar(nm, wt, nt_bc, ngs[:, i:i + 1],
                                        mybir.AluOpType.is_lt,
                                        mybir.AluOpType.mult)
                nc.vector.tensor_tensor(qslice(i), pos, nm,
                                        mybir.AluOpType.add)

            # stats for the pre-loaded strips (after quantize in FIFO order)
            for j in range(pre_strips):
                prep_compute(j)

            # ---- main loop -------------------------------------------------
            def chain_chunk(j, d, ps, qtr):
                for u in range(kq):
                    t = qtr * kq + u
                    nc.tensor.matmul(
                        ps, lhsT=xT_tiles[j][:, t, :],
                        rhs=qQs[qtr][:, u, d * 512:(d + 1) * 512],
                        start=(t == 0), stop=(t == kt - 1))

            def finish_psum_tile(j, d, ps):
                ob = outp.tile([128, 512], F32, tag="ob", bufs=4,
                               name=f"ob{j}_{d}")
                nc.vector.tensor_scalar(ob, ps, invb[:, j:j + 1], None,
                                        mybir.AluOpType.mult)
                nc.scalar.dma_start(
                    out=out_d[j * 128:(j + 1) * 128, d * 512:(d + 1) * 512],
                    in_=ob)

            def emit_psum_tile(j, d):
                ps = psum.tile([128, 512], F32, tag="mm", name=f"ps{j}_{d}")
                for qtr in range(n_kq):
                    chain_chunk(j, d, ps, qtr)
                finish_psum_tile(j, d, ps)

            # block 0: quarter-interleaved across all 8 banks, 2 waves
            for wave in range(2):
                tiles = [(j, d) for d in (2 * wave, 2 * wave + 1)
                         for j in range(strip_blk)]
                pss = {(j, d): psum.tile([128, 512], F32, tag="mm",
                                         name=f"ps{j}_{d}")
                       for (j, d) in tiles}
                for qtr in range(n_kq):
                    for (j, d) in tiles:
                        chain_chunk(j, d, pss[(j, d)], qtr)
                for (j, d) in tiles:
                    finish_psum_tile(j, d, pss[(j, d)])

            # blocks 1..: plain deep-pipelined chains, preps dripped in
            next_prep = pre_strips
            for b in range(1, n_blk):
                for d in range(n_wtile // 4):
                    for j in range(b * strip_blk, (b + 1) * strip_blk):
                        emit_psum_tile(j, d)
                    if next_prep < n_strip:
                        prep_dma(next_prep, pre=False)
                        prep_compute(next_prep)
                        next_prep += 1

    nc.compile()
    return nc


_NC_CACHE = {}


def _get_nc():
    if "nc" not in _NC_CACHE:
        _NC_CACHE["nc"] = build_nc()
    return _NC_CACHE["nc"]


def make_in_maps(x, weight, gamma):
    """Shard + lay out host-side. x:[B,S,K] f32, weight:[DOUT,K] f32."""
    x = np.asarray(x, dtype=np.float32)
    weight = np.ascontiguousarray(np.asarray(weight, dtype=np.float32))
    gamma = np.ascontiguousarray(np.asarray(gamma, dtype=np.float32))

    x16 = x.reshape(B * S, K).astype(np.float16)
    wT = np.ascontiguousarray(weight.T)  # [K, DOUT] f32
    in_maps = []
    for c in range(N_CORES):
        rg, cg = c // CG, c % CG
        in_maps.append({
            "x_sh": np.ascontiguousarray(
                x16[rg * TOK_SH:(rg + 1) * TOK_SH]),
            "w_shT": np.ascontiguousarray(
                wT[:, cg * DOUT_SH:(cg + 1) * DOUT_SH]),
            "w_red": weight[c * RED_ROWS:(c + 1) * RED_ROWS],
            "gamma": gamma,
        })
    return in_maps


def kernel(x, weight, gamma):
    in_maps = make_in_maps(x, weight, gamma)
    nc = _get_nc()
    res = run_bass_kernel_spmd(nc, in_maps, list(range(N_CORES))).results

    out = np.empty((B * S, DOUT), dtype=np.float32)
    for c in range(N_CORES):
        rg, cg = c // CG, c % CG
        out[rg * TOK_SH:(rg + 1) * TOK_SH,
            cg * DOUT_SH:(cg + 1) * DOUT_SH] = res[c]["out_sh"]
    return out.reshape(B, S, DOUT)
